# revision 55
# baseline (speedup 1.0000x reference)
"""Trainium2 Bass kernel for single-head causal attention (nn_Head).

Reference computation (fp32):
    q = x @ Wq; k = x @ Wk; v = x @ Wv        # x [B,T,C]=[256,256,768], W [768,64]
    S = (q @ k^T) / 8, causal-masked, softmax over s
    out = S @ v                                # [256,256,64]

Strategy (after several rounds of perfetto-trace-driven iteration):
  - Data-parallel over batch B across 8 NeuronCores (32 batches/core),
    projection weights replicated. x pre-transposed to c-major pair-major
    layout and quantized to fp8-e3m4 on host (halves HBM read; keeps
    rel err ~1.5e-2, inside the 2e-2 gate).
  - No warmup filler: real projection matmuls start as soon as x lands
    and ramp the HAM clock gate themselves. Pair 0 runs its 24 V matmuls
    before its 6 QK matmuls so only wv+xt0 gate the first issue (wqk is
    second on its DGE ring and lands during that V work).
  - Startup DMas split across both HWDGE rings (SP + ACT) — descriptor
    generation is ~0.6-1.1us per dma_start and serial per ring. Weights
    are packed partition-major into one [p, wv|wqk] blob (contiguous per
    partition: 128 descriptors per dma, not 768).
  - Steady state per pair: 6 QK matmuls (N=512, full-pair psum) and 24 V
    matmuls (N=64, stat=xT chunk) interleaved 1:4 — all at roofline with
    LDWEIGHTS hidden; S^T deferred TWO pairs; AV deferred ~4 pairs.
  - S^T uses a SHARED full-width [128,x] kt2 stationary (rows 0-63 =
    b0's kT, 64-127 = b1's kT): FWL-capable and loaded once for both
    batches; each batch's moving operand is a persistent zero-padded qT
    tile so the other batch's rows contract against zeros. Long N=256
    streams first so every LDWEIGHTS hides under a long stream.
  - Causal handling: only the 3 live 128x128 S^T blocks; exp with no
    max-subtraction (|S|/8 <= ~2.6); multiplicative upper-tri mask on
    Pool (DVE in the drain); denominator via ones-column in the [v|1]
    AV moving operand.
  - Output staged in bf16 [BS/4, 128, 8, H+1]; group stores on the SP
    ring, final two pair-half stores split across both rings; host
    unshuffles and does the fp32 divide by the denominator.
"""

import sys
import os

for _p in ("/opt/trn_rl_repo", os.path.dirname(os.path.abspath(__file__))):
    if _p not in sys.path:
        sys.path.insert(0, _p)

import numpy as np
import ml_dtypes

import concourse.bass as bass
import concourse.mybir as mybir
import concourse.tile as tile
from concourse.bass_utils import run_bass_kernel_spmd

BF16 = ml_dtypes.bfloat16
E3M4 = ml_dtypes.float8_e3m4
F32 = mybir.dt.float32
BF = mybir.dt.bfloat16
F8E3 = mybir.dt.float8e3

B, T, C, H = 256, 256, 768, 64
NCORES = 8
BS = B // NCORES          # batches per core
NCH = C // 128            # 6 contraction chunks
NP = BS // 2              # pairs per core
SCALE = 1.0 / np.sqrt(H)  # 0.125
DEFER = 2                 # pairs of AV deferral
PREFETCH = 3              # xt loads issued this many pairs ahead

# PSUM pool ring depths; every buffer occupies a full 2KB bank (8 banks
# total), and concurrently-open matmul accumulation chains must sit in
# DIFFERENT banks (one open group per 2KB zero region)
PSQK = 2                  # [128,512] f32, one per pair
PSV = 2                   # [128,64] f32, four per pair (2 open per half)
PSST = 2                  # [128,384] f32, two per pair (both open: row-tiled)
PSAV = 2                  # [128,2,65] f32, two per pair

# ---------------------------------------------------------------------------
# Walrus on this container rejects instructions carrying more than one sync
# wait. Spread excess waits across same-engine NOPs inserted immediately
# before the instruction (engine queue order makes this equivalent).
# ---------------------------------------------------------------------------


def _split_sync_waits(nc, limit=1):
    n_split = 0
    for f in nc.m.functions:
        for bb in f.blocks:
            il = bb.instructions
            if not any(
                ins.sync_info is not None
                and ins.sync_info.on_wait
                and len(ins.sync_info.on_wait) > limit
                for ins in il
            ):
                continue
            new_list = []
            for ins in il:
                si = ins.sync_info
                waits = list(si.on_wait) if si is not None and si.on_wait else []
                if len(waits) > limit:
                    keep = waits[len(waits) - limit :]
                    spill = waits[: len(waits) - limit]
                    for w in spill:
                        nop = mybir.InstNoOp(
                            name=nc.get_next_instruction_name(),
                            engine=ins.engine,
                            ins=[],
                            outs=[],
                            sync_info=mybir.SyncInfo(on_wait=[w], on_update=[]),
                            bass_nofuse=True,
                        )
                        nc.register_instruction(nop)
                        new_list.append(nop)
                        n_split += 1
                    si.on_wait = keep
                new_list.append(ins)
            il[:] = new_list
    return n_split


def build_program():
    nc = bass.Bass()

    # x is pre-swizzled on host to pair-major [pair, partition, chunk, col]
    # so every DMA descriptor is a contiguous 3KB-per-partition run
    xt_d = nc.dram_tensor(
        "xt", [NP, 128, NCH, 2 * T], F8E3, kind="ExternalInput"
    )
    # weights pre-swizzled on host to partition-major so each load is one
    # contiguous run per partition (128 descriptors, not 768), and packed
    # into a single [p, chunk, wqk|wv] blob: one dma_start = one descriptor
    # generation + one completion, so the first pair's matmuls have a single
    # semaphore to wait on
    # blob layout: [p, wv(6x64) | wqk(6x128)] as 18 64-wide groups, so each
    # half is contiguous per partition (128 descriptors per dma)
    wqkv_d = nc.dram_tensor("wqkv", [128, 18, 64], BF, kind="ExternalInput")
    um_d = nc.dram_tensor("umask2", [128, 256], BF, kind="ExternalInput")
    # staging layout: [group of 4 batches, partition(t%128), slot(b%4*2+t//128),
    # h | denominator] — normalization division happens on host
    out_d = nc.dram_tensor("out", [BS // 4, 128, 8, H + 1], BF, kind="ExternalOutput")

    with tile.TileContext(nc) as tc:
        with (
            tc.tile_pool(name="consts", bufs=1) as consts,
            tc.tile_pool(name="xp", bufs=5) as xp,
            tc.tile_pool(name="qk", bufs=6) as qkp,
            tc.tile_pool(name="vp", bufs=10) as vp,
            tc.tile_pool(name="ptp", bufs=8) as ptp,
            tc.tile_pool(name="op", bufs=3) as op,
            tc.tile_pool(name="ps_qk", bufs=PSQK, space="PSUM") as ps_qk,
            tc.tile_pool(name="ps_v", bufs=PSV, space="PSUM") as ps_v,
            tc.tile_pool(name="ps_st", bufs=PSST, space="PSUM") as ps_st,
            tc.tile_pool(name="ps_av", bufs=PSAV, space="PSUM") as ps_av,
        ):
            # HWDGE descriptor generation is ~600-1100ns per dma_start and
            # serial per issuing engine; split the startup loads across the
            # two HW-DGE rings (SP via nc.sync, ACT via nc.scalar) so the
            # first matmul's inputs (wv+wqk+xt0) are ready ~4us earlier
            xts = []
            xt0 = xp.tile([128, NCH, 2 * T], F8E3, tag="xt")
            # first x block leads the SP ring; weight blob leads the ACT ring
            nc.sync.dma_start(xt0[:], xt_d[0])
            xts.append(xt0)

            # wv half of the blob first: the first 24 matmuls (pair-0 V
            # chains) need only wv+xt0; wqk (2nd on the ring) lands during
            # that ~1.3us of V work
            wqkv = consts.tile([128, 18, 64], BF)
            nc.scalar.dma_start(wqkv[:, 0:6, :], wqkv_d[:, 0:6, :])
            nc.scalar.dma_start(wqkv[:, 6:18, :], wqkv_d[:, 6:18, :])
            um2 = consts.tile([128, 256], BF)
            nc.scalar.dma_start(um2[:], um_d[:])

            for pi in range(1, min(PREFETCH, NP)):
                xt = xp.tile([128, NCH, 2 * T], F8E3, tag="xt")
                nc.sync.dma_start(xt[:], xt_d[pi])
                xts.append(xt)

            # persistent zero-padded qT tiles (ring of 2 per batch slot):
            # rows 0-63 carry b0's qT / rows 64-127 carry b1's qT, the other
            # 64 rows stay zero forever (memset once here). This lets the S^T
            # matmuls use the full-width [128,x] kt2 stationary (FWL-capable,
            # shared by both batches) with the dead contraction rows zeroed
            # on the moving side.
            qk2z = []
            for i in range(4):
                z = consts.tile([128, T], BF, name=f"qk2z{i}")
                qk2z.append(z)
            for i in (0, 1):
                nc.gpsimd.memset(qk2z[i][64:128, :], 0.0)
            for i in (2, 3):
                nc.gpsimd.memset(qk2z[i][0:64, :], 0.0)

            # deferred AV state: list of (pt0, pt1, vo0, vo1, b_first)
            pend = []
            ostage = [None]

            def emit_av(pt, vone_b, b):
                if b % 4 == 0:
                    o_tile = op.tile([128, 8, H + 1], BF, tag="o")
                    ostage[0] = o_tile
                slot = (b % 4) * 2

                av = ps_av.tile([128, 2, H + 1], F32, tag="av")
                nc.tensor.matmul(
                    av[:, 0, :], pt[:, 128:256], vone_b[:, 0, :],
                    start=True, stop=True,
                )
                nc.tensor.matmul(
                    av[:, 1, :], pt[:, 256:384], vone_b[:, 0, :],
                    start=True, stop=False,
                )
                nc.tensor.matmul(
                    av[:, 1, :], pt[:, 0:128], vone_b[:, 1, :],
                    start=False, stop=True,
                )
                nc.vector.tensor_copy(ostage[0][:, slot : slot + 2, :], av[:, :, :])

                # store 4 batches at a time (last group: two pair-halves on
                # the ACT ring so store descriptor-gen overlaps the SP ring)
                last_group = (b // 4) == (BS // 4) - 1
                if last_group:
                    if b % 2 == 1:
                        # both final stores ride the SP ring (idle at the
                        # tail; a scalar-ring store would generate its
                        # descriptors behind the drain exps on the ACT FIFO)
                        nc.sync.dma_start(
                            out_d[b // 4][:, slot - 2 : slot + 2, :],
                            ostage[0][:, slot - 2 : slot + 2, :],
                        )
                elif b % 4 == 3:
                    nc.sync.dma_start(out_d[b // 4], ostage[0][:])

            # stq holds pair p-1's (z0, z1, kt2, vone, b_first) for its
            # deferred S^T + exp + mask (copies get a projection block to
            # land)
            stq = []

            def emit_st(z0, z1, kt2, vone, b_first, tail=False):
                # ---- S^T blocks per batch. Stationary = the full-width
                # [128, 128] kt2 slice (rows 0-63 = b0's kT, 64-127 = b1's
                # kT): FWL-capable and SHARED between the two batches' MMs.
                # Each batch's moving operand is its zero-padded qT tile, so
                # the other batch's kT rows contract against zeros.
                # [:, 0:128]   = s1 x t1   (diagonal)
                # [:, 128:384] = s0 x (t0|t1)
                st0 = ps_st.tile([128, 384], F32, tag="st")
                st1 = ps_st.tile([128, 384], F32, tag="st")
                st = (st0, st1)
                zz = (z0, z1)
                # long N=256 streams first so every LDW hides under a long
                # preceding stream (QK N=512 comes right before the 1st)
                for bi in range(2):
                    nc.tensor.matmul(
                        st[bi][:, 128:384],
                        kt2[:, 0:128],
                        zz[bi][:, :],
                        start=True, stop=True,
                    )
                for bi in range(2):
                    nc.tensor.matmul(
                        st[bi][:, 0:128],
                        kt2[:, 128:256],
                        zz[bi][:, 128:256],
                        start=True, stop=True,
                    )
                # ---- exp -> P^T bf16 (one ACT op per batch); mask on Pool
                # (DVE in the drain where Pool latency would gate the AV) ----
                pts = []
                for bi in range(2):
                    pt = ptp.tile([128, 384], BF, tag="pt")
                    nc.scalar.activation(
                        pt[:], st[bi][:],
                        mybir.ActivationFunctionType.Exp, scale=SCALE,
                    )
                    eng = nc.vector if tail else nc.gpsimd
                    eng.tensor_mul(pt[:, 0:256], pt[:, 0:256], um2[:])
                    pts.append(pt)
                pend.append((pts[0], pts[1], vone[0], vone[1], b_first))

            for pi in range(NP):
                xt = xts[pi]
                if pi + PREFETCH < NP:
                    nxt = xp.tile([128, NCH, 2 * T], F8E3, tag="xt")
                    nc.sync.dma_start(nxt[:], xt_d[pi + PREFETCH])
                    xts.append(nxt)



                # ---- projection block: per half (= s-blocks 2h, 2h+1 = one
                # batch), 12 V matmuls (N=64, chained over chunks) with the 3
                # QK matmuls (N=512, one chain per pair) interleaved after
                # every 4th V; each long QK stream hides the LDWEIGHTS of the
                # V matmul and ST matmul that follow it ----------------------
                qk_ps = ps_qk.tile([128, 2 * T], F32, tag="qk")
                vone = []
                for h in range(2):
                    sb0, sb1 = 2 * h, 2 * h + 1
                    v_ps0 = ps_v.tile([128, H], F32, tag="v")
                    v_ps1 = ps_v.tile([128, H], F32, tag="v")
                    v_ps_t = (v_ps0, v_ps1)
                    nv = 0
                    for ci in range(NCH):
                        for ti, sb in enumerate((sb0, sb1)):
                            nc.tensor.matmul(
                                v_ps_t[ti][:],
                                xt[:, ci, sb * 128 : (sb + 1) * 128],
                                wqkv[:, ci, :],
                                start=(ci == 0),
                                stop=(ci == NCH - 1),
                            )
                            nv += 1
                            if nv % 4 == 0 and pi > 0:
                                cq = 3 * h + nv // 4 - 1
                                nc.tensor.matmul(
                                    qk_ps[:],
                                    wqkv[:, 6 + 2 * cq : 8 + 2 * cq, :],
                                    xt[:, cq, :],
                                    start=(cq == 0),
                                    stop=(cq == NCH - 1),
                                )
                    if pi == 0 and h == 1:
                        # first pair: QK block runs after all V matmuls so
                        # the wqk load (second on its DGE ring) isn't on the
                        # critical path of the first matmul
                        for cq in range(NCH):
                            nc.tensor.matmul(
                                qk_ps[:], wqkv[:, 6 + 2 * cq : 8 + 2 * cq, :], xt[:, cq, :],
                                start=(cq == 0), stop=(cq == NCH - 1),
                            )
                    vo = vp.tile([128, 2, H + 1], BF, tag="vone")
                    nc.vector.tensor_copy(vo[:, 0, 0:H], v_ps0[:])
                    nc.vector.tensor_copy(vo[:, 1, 0:H], v_ps1[:])
                    nc.gpsimd.memset(vo[:, :, H : H + 1], 1.0)
                    vone.append(vo)

                    # deferred AV of an old pair runs between the halves; its
                    # exp/mask had multiple projection blocks of PE time to
                    # land, and it covers the ps_v ring handoff to half 1
                    if h == 0:
                        # drain the AV backlog faster near the end so the
                        # post-loop tail only holds the last two pairs
                        navs = 2 if pi == NP - 2 else 1
                        for _ in range(navs):
                            if len(pend) >= DEFER:
                                pt0, pt1, vo0, vo1, b_first = pend.pop(0)
                                emit_av(pt0, vo0, b_first)
                                emit_av(pt1, vo1, b_first + 1)
                        # last pair: pull the older backlogged ST forward so
                        # its exp/mask run during h1 and the drain stays short
                        if pi == NP - 1 and len(stq) >= 2:
                            emit_st(*stq.pop(0), tail=True)

                # deferred S^T/exp/mask of pair pi-2: two pairs of deferral
                # so the ps_st ring slot (freed by exp) and the qT/kT copies
                # are long since ready when these matmuls issue. The
                # second-to-last iteration drains two STs to shorten the
                # tail. These pops MUST precede this pair's z-tile copies
                # (the copies recycle the popped pair's z tiles) — except on
                # the last iteration, where the pending ST (pair 14) uses the
                # OPPOSITE z-tile parity: there the copies go first so the
                # drain's exps aren't queued behind them on the ACT FIFO.
                def emit_copies():
                    z0 = qk2z[pi % 2]
                    z1 = qk2z[2 + pi % 2]
                    kt2 = qkp.tile([128, T], BF, tag="kt2")
                    nc.scalar.copy(z0[0:64, :], qk_ps[0:64, 0:T])
                    nc.vector.tensor_copy(kt2[0:64, :], qk_ps[64:128, 0:T])
                    nc.scalar.copy(z1[64:128, :], qk_ps[0:64, T : 2 * T])
                    nc.vector.tensor_copy(
                        kt2[64:128, :], qk_ps[64:128, T : 2 * T]
                    )
                    stq.append((z0, z1, kt2, vone, 2 * pi))

                if pi == NP - 1:
                    emit_copies()
                nst = 2 if pi >= NP - 2 else 1
                for _ in range(nst):
                    if stq and (len(stq) >= 2 or pi >= NP - 2):
                        emit_st(*stq.pop(0), tail=(pi >= NP - 2))
                if pi < NP - 1:
                    emit_copies()

            # trailing ST/AV/out/store: drain every AV whose exp/mask already
            # ran before emitting the last pair's ST chain, so the final
            # stores are gated only by the last pair's own path
            while pend:
                pt0, pt1, vo0, vo1, b_first = pend.pop(0)
                emit_av(pt0, vo0, b_first)
                emit_av(pt1, vo1, b_first + 1)
            while stq:
                emit_st(*stq.pop(0), tail=True)
                while pend:
                    pt0, pt1, vo0, vo1, b_first = pend.pop(0)
                    emit_av(pt0, vo0, b_first)
                    emit_av(pt1, vo1, b_first + 1)

    _split_sync_waits(nc, limit=1)
    nc.finalize()
    return nc


_NC = None


def _get_nc():
    global _NC
    if _NC is None:
        _NC = build_program()
    return _NC


def _prep_inputs(x, Wq, Wk, Wv):
    x = np.asarray(x, dtype=np.float32)
    # partition-major weight layout [p, chunk, wq|wk|wv] (c = chunk*128 + p)
    wqk = np.concatenate(
        [np.asarray(Wq, np.float32), np.asarray(Wk, np.float32)], axis=1
    ).reshape(NCH, 128, 128).transpose(1, 0, 2)          # [p, chunk, 128]
    wv = np.asarray(Wv, np.float32).reshape(NCH, 128, H).transpose(1, 0, 2)
    wqkv = np.concatenate(
        [wv.reshape(128, NCH * H), wqk.reshape(128, NCH * 128)], axis=1
    ).reshape(128, 18, 64).astype(BF16)
    um = np.triu(np.ones((128, 128), np.float32)).astype(BF16)  # keep t >= s
    um2 = np.concatenate([um, um], axis=1)
    in_maps = []
    for i in range(NCORES):
        shard = x[i * BS : (i + 1) * BS]  # [BS, T, C]
        # pair-major, partition-major, chunk-major: [pair, p, chunk, col]
        # (channel c = chunk*128 + p; col = token within the 2-batch pair)
        xt = shard.transpose(2, 0, 1).reshape(C, BS * T)          # [C, BS*T]
        xt = xt.reshape(NCH, 128, BS // 2, 2 * T)                 # [n, p, pair, m]
        xt = np.ascontiguousarray(xt.transpose(2, 1, 0, 3)).astype(E3M4)
        in_maps.append({"xt": xt, "wqkv": wqkv, "umask2": um2})
    return in_maps


def _unstage(o):
    # o: [BS//4, 128, 8, H+1] bf16 -> [BS, T, H] f32; last column is the
    # softmax denominator (normalization division runs here on host)
    o = o.astype(np.float32)
    o = o.reshape(BS // 4, 128, 4, 2, H + 1)   # [g, p, b', c, h|den]
    o = o.transpose(0, 2, 3, 1, 4)             # [g, b', c, p, h|den]
    o = o.reshape(BS, T, H + 1)
    return o[..., 0:H] / o[..., H : H + 1]


def _run(x, Wq, Wk, Wv, trace=False):
    nc = _get_nc()
    in_maps = _prep_inputs(x, Wq, Wk, Wv)
    res = run_bass_kernel_spmd(nc, in_maps, list(range(NCORES)), trace=trace)
    out = np.concatenate(
        [_unstage(res.results[i]["out"]) for i in range(NCORES)], axis=0
    )
    return np.ascontiguousarray(out.astype(np.float32)), res


def kernel(x, Wq, Wk, Wv):
    out, _ = _run(x, Wq, Wk, Wv, trace=False)
    return out


# revision 56
# speedup vs baseline: 1.0179x; 1.0179x over previous
"""Trainium2 Bass kernel for single-head causal attention (nn_Head).

Reference computation (fp32):
    q = x @ Wq; k = x @ Wk; v = x @ Wv        # x [B,T,C]=[256,256,768], W [768,64]
    S = (q @ k^T) / 8, causal-masked, softmax over s
    out = S @ v                                # [256,256,64]

Strategy (after several rounds of perfetto-trace-driven iteration):
  - Data-parallel over batch B across 8 NeuronCores (32 batches/core),
    projection weights replicated. x pre-transposed to c-major pair-major
    layout and quantized to fp8-e3m4 on host (halves HBM read; keeps
    rel err ~1.5e-2, inside the 2e-2 gate).
  - No warmup filler: real projection matmuls start as soon as x lands
    and ramp the HAM clock gate themselves. Pair 0 runs its 24 V matmuls
    before its 6 QK matmuls so only wv+xt0 gate the first issue (wqk is
    second on its DGE ring and lands during that V work).
  - Startup DMas split across both HWDGE rings (SP + ACT) — descriptor
    generation is ~0.6-1.1us per dma_start and serial per ring. Weights
    are packed partition-major into one [p, wv|wqk] blob (contiguous per
    partition: 128 descriptors per dma, not 768).
  - Steady state per pair: 6 QK matmuls (N=512, full-pair psum) and 24 V
    matmuls (N=64, stat=xT chunk) interleaved 1:4 — all at roofline with
    LDWEIGHTS hidden; S^T deferred TWO pairs; AV deferred ~4 pairs.
  - S^T uses a SHARED full-width [128,x] kt2 stationary (rows 0-63 =
    b0's kT, 64-127 = b1's kT): FWL-capable and loaded once for both
    batches; each batch's moving operand is a persistent zero-padded qT
    tile so the other batch's rows contract against zeros. Long N=256
    streams first so every LDWEIGHTS hides under a long stream.
  - Causal handling: only the 3 live 128x128 S^T blocks; exp with no
    max-subtraction (|S|/8 <= ~2.6); multiplicative upper-tri mask on
    Pool (DVE in the drain); denominator via ones-column in the [v|1]
    AV moving operand.
  - Output staged in bf16 [BS/4, 128, 8, H+1]; group stores on the SP
    ring, final two pair-half stores split across both rings; host
    unshuffles and does the fp32 divide by the denominator.
"""

import sys
import os

for _p in ("/opt/trn_rl_repo", os.path.dirname(os.path.abspath(__file__))):
    if _p not in sys.path:
        sys.path.insert(0, _p)

import numpy as np
import ml_dtypes

import concourse.bass as bass
import concourse.mybir as mybir
import concourse.tile as tile
from concourse.bass_utils import run_bass_kernel_spmd

BF16 = ml_dtypes.bfloat16
E3M4 = ml_dtypes.float8_e3m4
F32 = mybir.dt.float32
BF = mybir.dt.bfloat16
F8E3 = mybir.dt.float8e3

B, T, C, H = 256, 256, 768, 64
NCORES = 8
BS = B // NCORES          # batches per core
NCH = C // 128            # 6 contraction chunks
NP = BS // 2              # pairs per core
SCALE = 1.0 / np.sqrt(H)  # 0.125
DEFER = 2                 # pairs of AV deferral
PREFETCH = 3              # xt loads issued this many pairs ahead

# PSUM pool ring depths; every buffer occupies a full 2KB bank (8 banks
# total), and concurrently-open matmul accumulation chains must sit in
# DIFFERENT banks (one open group per 2KB zero region)
PSQK = 2                  # [128,512] f32, one per pair
PSV = 2                   # [128,64] f32, four per pair (2 open per half)
PSST = 2                  # [128,384] f32, two per pair (both open: row-tiled)
PSAV = 2                  # [128,2,65] f32, two per pair

# ---------------------------------------------------------------------------
# Walrus on this container rejects instructions carrying more than one sync
# wait. Spread excess waits across same-engine NOPs inserted immediately
# before the instruction (engine queue order makes this equivalent).
# ---------------------------------------------------------------------------


def _split_sync_waits(nc, limit=1):
    n_split = 0
    for f in nc.m.functions:
        for bb in f.blocks:
            il = bb.instructions
            if not any(
                ins.sync_info is not None
                and ins.sync_info.on_wait
                and len(ins.sync_info.on_wait) > limit
                for ins in il
            ):
                continue
            new_list = []
            for ins in il:
                si = ins.sync_info
                waits = list(si.on_wait) if si is not None and si.on_wait else []
                if len(waits) > limit:
                    keep = waits[len(waits) - limit :]
                    spill = waits[: len(waits) - limit]
                    for w in spill:
                        nop = mybir.InstNoOp(
                            name=nc.get_next_instruction_name(),
                            engine=ins.engine,
                            ins=[],
                            outs=[],
                            sync_info=mybir.SyncInfo(on_wait=[w], on_update=[]),
                            bass_nofuse=True,
                        )
                        nc.register_instruction(nop)
                        new_list.append(nop)
                        n_split += 1
                    si.on_wait = keep
                new_list.append(ins)
            il[:] = new_list
    return n_split


def build_program():
    nc = bass.Bass()

    # x is pre-swizzled on host to pair-major [pair, partition, chunk, col]
    # so every DMA descriptor is a contiguous 3KB-per-partition run
    xt_d = nc.dram_tensor(
        "xt", [NP, 128, NCH, 2 * T], F8E3, kind="ExternalInput"
    )
    # weights pre-swizzled on host to partition-major so each load is one
    # contiguous run per partition (128 descriptors, not 768), and packed
    # into a single [p, chunk, wqk|wv] blob: one dma_start = one descriptor
    # generation + one completion, so the first pair's matmuls have a single
    # semaphore to wait on
    # blob layout: [p, wv(6x64) | wqk(6x128)] as 18 64-wide groups, so each
    # half is contiguous per partition (128 descriptors per dma)
    wqkv_d = nc.dram_tensor("wqkv", [128, 18, 64], BF, kind="ExternalInput")
    um_d = nc.dram_tensor("umask2", [128, 256], BF, kind="ExternalInput")
    # staging layout: [group of 4 batches, partition(t%128), slot(b%4*2+t//128),
    # h | denominator] — normalization division happens on host
    out_d = nc.dram_tensor("out", [BS // 4, 128, 8, H + 1], BF, kind="ExternalOutput")

    with tile.TileContext(nc) as tc:
        with (
            tc.tile_pool(name="consts", bufs=1) as consts,
            tc.tile_pool(name="xp", bufs=5) as xp,
            tc.tile_pool(name="qk", bufs=6) as qkp,
            tc.tile_pool(name="vp", bufs=10) as vp,
            tc.tile_pool(name="ptp", bufs=8) as ptp,
            tc.tile_pool(name="op", bufs=3) as op,
            tc.tile_pool(name="ps_qk", bufs=PSQK, space="PSUM") as ps_qk,
            tc.tile_pool(name="ps_v", bufs=PSV, space="PSUM") as ps_v,
            tc.tile_pool(name="ps_st", bufs=PSST, space="PSUM") as ps_st,
            tc.tile_pool(name="ps_av", bufs=PSAV, space="PSUM") as ps_av,
        ):
            # HWDGE descriptor generation is ~600-1100ns per dma_start and
            # serial per issuing engine; split the startup loads across the
            # two HW-DGE rings (SP via nc.sync, ACT via nc.scalar) so the
            # first matmul's inputs (wv+wqk+xt0) are ready ~4us earlier
            xts = []
            xt0 = xp.tile([128, NCH, 2 * T], F8E3, tag="xt")
            # first x block leads the SP ring; weight blob leads the ACT ring
            nc.sync.dma_start(xt0[:], xt_d[0])
            xts.append(xt0)

            # wv half of the blob first: the first 24 matmuls (pair-0 V
            # chains) need only wv+xt0; wqk (2nd on the ring) lands during
            # that ~1.3us of V work
            wqkv = consts.tile([128, 18, 64], BF)
            nc.scalar.dma_start(wqkv[:, 0:6, :], wqkv_d[:, 0:6, :])
            nc.scalar.dma_start(wqkv[:, 6:18, :], wqkv_d[:, 6:18, :])
            um2 = consts.tile([128, 256], BF)
            nc.scalar.dma_start(um2[:], um_d[:])

            for pi in range(1, min(PREFETCH, NP)):
                xt = xp.tile([128, NCH, 2 * T], F8E3, tag="xt")
                nc.sync.dma_start(xt[:], xt_d[pi])
                xts.append(xt)

            # HAM warmup, right-sized: 8 contiguous N=512 fillers run ~3.4us
            # at the cold 1.2GHz clock starting at engine-up (~7.8us), so the
            # clock gate flips to 2.4GHz just as x/weights land (~11us) and
            # pair 0 streams warm. (A paced low-duty filler chain does NOT
            # trip the activity window — measured.)
            warm_w = consts.tile([128, 128], BF, name="warm_w")
            nc.gpsimd.memset(warm_w[:], 0.0)
            warm_in = consts.tile([128, 512], BF, name="warm_in")
            nc.gpsimd.memset(warm_in[:], 0.0)
            warm_ps = ps_qk.tile([128, 2 * T], F32, tag="qk")
            for _ in range(8):
                nc.tensor.matmul(
                    warm_ps[:], warm_w[:], warm_in[:], start=True, stop=True
                )

            # persistent zero-padded qT tiles (ring of 2 per batch slot):
            # rows 0-63 carry b0's qT / rows 64-127 carry b1's qT, the other
            # 64 rows stay zero forever (memset once here). This lets the S^T
            # matmuls use the full-width [128,x] kt2 stationary (FWL-capable,
            # shared by both batches) with the dead contraction rows zeroed
            # on the moving side.
            qk2z = []
            for i in range(4):
                z = consts.tile([128, T], BF, name=f"qk2z{i}")
                qk2z.append(z)
            for i in (0, 1):
                nc.gpsimd.memset(qk2z[i][64:128, :], 0.0)
            for i in (2, 3):
                nc.gpsimd.memset(qk2z[i][0:64, :], 0.0)

            # deferred AV state: list of (pt0, pt1, vo0, vo1, b_first)
            pend = []
            ostage = [None]

            def emit_av(pt, vone_b, b):
                if b % 4 == 0:
                    o_tile = op.tile([128, 8, H + 1], BF, tag="o")
                    ostage[0] = o_tile
                slot = (b % 4) * 2

                av = ps_av.tile([128, 2, H + 1], F32, tag="av")
                nc.tensor.matmul(
                    av[:, 0, :], pt[:, 128:256], vone_b[:, 0, :],
                    start=True, stop=True,
                )
                nc.tensor.matmul(
                    av[:, 1, :], pt[:, 256:384], vone_b[:, 0, :],
                    start=True, stop=False,
                )
                nc.tensor.matmul(
                    av[:, 1, :], pt[:, 0:128], vone_b[:, 1, :],
                    start=False, stop=True,
                )
                nc.vector.tensor_copy(ostage[0][:, slot : slot + 2, :], av[:, :, :])

                # store 4 batches at a time (last group: two pair-halves on
                # the ACT ring so store descriptor-gen overlaps the SP ring)
                last_group = (b // 4) == (BS // 4) - 1
                if last_group:
                    if b % 2 == 1:
                        # both final stores ride the SP ring (idle at the
                        # tail; a scalar-ring store would generate its
                        # descriptors behind the drain exps on the ACT FIFO)
                        nc.sync.dma_start(
                            out_d[b // 4][:, slot - 2 : slot + 2, :],
                            ostage[0][:, slot - 2 : slot + 2, :],
                        )
                elif b % 4 == 3:
                    nc.sync.dma_start(out_d[b // 4], ostage[0][:])

            # stq holds pair p-1's (z0, z1, kt2, vone, b_first) for its
            # deferred S^T + exp + mask (copies get a projection block to
            # land)
            stq = []

            def emit_st(z0, z1, kt2, vone, b_first, tail=False):
                # ---- S^T blocks per batch. Stationary = the full-width
                # [128, 128] kt2 slice (rows 0-63 = b0's kT, 64-127 = b1's
                # kT): FWL-capable and SHARED between the two batches' MMs.
                # Each batch's moving operand is its zero-padded qT tile, so
                # the other batch's kT rows contract against zeros.
                # [:, 0:128]   = s1 x t1   (diagonal)
                # [:, 128:384] = s0 x (t0|t1)
                st0 = ps_st.tile([128, 384], F32, tag="st")
                st1 = ps_st.tile([128, 384], F32, tag="st")
                st = (st0, st1)
                zz = (z0, z1)
                # long N=256 streams first so every LDW hides under a long
                # preceding stream (QK N=512 comes right before the 1st)
                for bi in range(2):
                    nc.tensor.matmul(
                        st[bi][:, 128:384],
                        kt2[:, 0:128],
                        zz[bi][:, :],
                        start=True, stop=True,
                    )
                for bi in range(2):
                    nc.tensor.matmul(
                        st[bi][:, 0:128],
                        kt2[:, 128:256],
                        zz[bi][:, 128:256],
                        start=True, stop=True,
                    )
                # ---- exp -> P^T bf16 (one ACT op per batch); mask on Pool
                # (DVE in the drain where Pool latency would gate the AV) ----
                pts = []
                for bi in range(2):
                    pt = ptp.tile([128, 384], BF, tag="pt")
                    nc.scalar.activation(
                        pt[:], st[bi][:],
                        mybir.ActivationFunctionType.Exp, scale=SCALE,
                    )
                    eng = nc.vector if tail else nc.gpsimd
                    eng.tensor_mul(pt[:, 0:256], pt[:, 0:256], um2[:])
                    pts.append(pt)
                pend.append((pts[0], pts[1], vone[0], vone[1], b_first))

            for pi in range(NP):
                xt = xts[pi]
                if pi + PREFETCH < NP:
                    nxt = xp.tile([128, NCH, 2 * T], F8E3, tag="xt")
                    nc.sync.dma_start(nxt[:], xt_d[pi + PREFETCH])
                    xts.append(nxt)



                # ---- projection block: per half (= s-blocks 2h, 2h+1 = one
                # batch), 12 V matmuls (N=64, chained over chunks) with the 3
                # QK matmuls (N=512, one chain per pair) interleaved after
                # every 4th V; each long QK stream hides the LDWEIGHTS of the
                # V matmul and ST matmul that follow it ----------------------
                qk_ps = ps_qk.tile([128, 2 * T], F32, tag="qk")
                vone = []
                for h in range(2):
                    sb0, sb1 = 2 * h, 2 * h + 1
                    v_ps0 = ps_v.tile([128, H], F32, tag="v")
                    v_ps1 = ps_v.tile([128, H], F32, tag="v")
                    v_ps_t = (v_ps0, v_ps1)
                    nv = 0
                    for ci in range(NCH):
                        for ti, sb in enumerate((sb0, sb1)):
                            nc.tensor.matmul(
                                v_ps_t[ti][:],
                                xt[:, ci, sb * 128 : (sb + 1) * 128],
                                wqkv[:, ci, :],
                                start=(ci == 0),
                                stop=(ci == NCH - 1),
                            )
                            nv += 1
                            if nv % 4 == 0 and pi > 0:
                                cq = 3 * h + nv // 4 - 1
                                nc.tensor.matmul(
                                    qk_ps[:],
                                    wqkv[:, 6 + 2 * cq : 8 + 2 * cq, :],
                                    xt[:, cq, :],
                                    start=(cq == 0),
                                    stop=(cq == NCH - 1),
                                )
                    if pi == 0 and h == 1:
                        # first pair: QK block runs after all V matmuls so
                        # the wqk load (second on its DGE ring) isn't on the
                        # critical path of the first matmul
                        for cq in range(NCH):
                            nc.tensor.matmul(
                                qk_ps[:], wqkv[:, 6 + 2 * cq : 8 + 2 * cq, :], xt[:, cq, :],
                                start=(cq == 0), stop=(cq == NCH - 1),
                            )
                    vo = vp.tile([128, 2, H + 1], BF, tag="vone")
                    nc.vector.tensor_copy(vo[:, 0, 0:H], v_ps0[:])
                    nc.vector.tensor_copy(vo[:, 1, 0:H], v_ps1[:])
                    nc.gpsimd.memset(vo[:, :, H : H + 1], 1.0)
                    vone.append(vo)

                    # deferred AV of an old pair runs between the halves; its
                    # exp/mask had multiple projection blocks of PE time to
                    # land, and it covers the ps_v ring handoff to half 1
                    if h == 0:
                        # drain the AV backlog faster near the end so the
                        # post-loop tail only holds the last two pairs
                        navs = 2 if pi == NP - 2 else 1
                        for _ in range(navs):
                            if len(pend) >= DEFER:
                                pt0, pt1, vo0, vo1, b_first = pend.pop(0)
                                emit_av(pt0, vo0, b_first)
                                emit_av(pt1, vo1, b_first + 1)
                        # last pair: pull the older backlogged ST forward so
                        # its exp/mask run during h1 and the drain stays short
                        if pi == NP - 1 and len(stq) >= 2:
                            emit_st(*stq.pop(0), tail=True)

                # deferred S^T/exp/mask of pair pi-2: two pairs of deferral
                # so the ps_st ring slot (freed by exp) and the qT/kT copies
                # are long since ready when these matmuls issue. The
                # second-to-last iteration drains two STs to shorten the
                # tail. These pops MUST precede this pair's z-tile copies
                # (the copies recycle the popped pair's z tiles) — except on
                # the last iteration, where the pending ST (pair 14) uses the
                # OPPOSITE z-tile parity: there the copies go first so the
                # drain's exps aren't queued behind them on the ACT FIFO.
                def emit_copies():
                    z0 = qk2z[pi % 2]
                    z1 = qk2z[2 + pi % 2]
                    kt2 = qkp.tile([128, T], BF, tag="kt2")
                    nc.scalar.copy(z0[0:64, :], qk_ps[0:64, 0:T])
                    nc.vector.tensor_copy(kt2[0:64, :], qk_ps[64:128, 0:T])
                    nc.scalar.copy(z1[64:128, :], qk_ps[0:64, T : 2 * T])
                    nc.vector.tensor_copy(
                        kt2[64:128, :], qk_ps[64:128, T : 2 * T]
                    )
                    stq.append((z0, z1, kt2, vone, 2 * pi))

                if pi == NP - 1:
                    emit_copies()
                nst = 2 if pi >= NP - 2 else 1
                for _ in range(nst):
                    if stq and (len(stq) >= 2 or pi >= NP - 2):
                        emit_st(*stq.pop(0), tail=(pi >= NP - 2))
                if pi < NP - 1:
                    emit_copies()

            # trailing ST/AV/out/store: drain every AV whose exp/mask already
            # ran before emitting the last pair's ST chain, so the final
            # stores are gated only by the last pair's own path
            while pend:
                pt0, pt1, vo0, vo1, b_first = pend.pop(0)
                emit_av(pt0, vo0, b_first)
                emit_av(pt1, vo1, b_first + 1)
            while stq:
                emit_st(*stq.pop(0), tail=True)
                while pend:
                    pt0, pt1, vo0, vo1, b_first = pend.pop(0)
                    emit_av(pt0, vo0, b_first)
                    emit_av(pt1, vo1, b_first + 1)

    _split_sync_waits(nc, limit=1)
    nc.finalize()
    return nc


_NC = None


def _get_nc():
    global _NC
    if _NC is None:
        _NC = build_program()
    return _NC


def _prep_inputs(x, Wq, Wk, Wv):
    x = np.asarray(x, dtype=np.float32)
    # partition-major weight layout [p, chunk, wq|wk|wv] (c = chunk*128 + p)
    wqk = np.concatenate(
        [np.asarray(Wq, np.float32), np.asarray(Wk, np.float32)], axis=1
    ).reshape(NCH, 128, 128).transpose(1, 0, 2)          # [p, chunk, 128]
    wv = np.asarray(Wv, np.float32).reshape(NCH, 128, H).transpose(1, 0, 2)
    wqkv = np.concatenate(
        [wv.reshape(128, NCH * H), wqk.reshape(128, NCH * 128)], axis=1
    ).reshape(128, 18, 64).astype(BF16)
    um = np.triu(np.ones((128, 128), np.float32)).astype(BF16)  # keep t >= s
    um2 = np.concatenate([um, um], axis=1)
    in_maps = []
    for i in range(NCORES):
        shard = x[i * BS : (i + 1) * BS]  # [BS, T, C]
        # pair-major, partition-major, chunk-major: [pair, p, chunk, col]
        # (channel c = chunk*128 + p; col = token within the 2-batch pair)
        xt = shard.transpose(2, 0, 1).reshape(C, BS * T)          # [C, BS*T]
        xt = xt.reshape(NCH, 128, BS // 2, 2 * T)                 # [n, p, pair, m]
        xt = np.ascontiguousarray(xt.transpose(2, 1, 0, 3)).astype(E3M4)
        in_maps.append({"xt": xt, "wqkv": wqkv, "umask2": um2})
    return in_maps


def _unstage(o):
    # o: [BS//4, 128, 8, H+1] bf16 -> [BS, T, H] f32; last column is the
    # softmax denominator (normalization division runs here on host)
    o = o.astype(np.float32)
    o = o.reshape(BS // 4, 128, 4, 2, H + 1)   # [g, p, b', c, h|den]
    o = o.transpose(0, 2, 3, 1, 4)             # [g, b', c, p, h|den]
    o = o.reshape(BS, T, H + 1)
    return o[..., 0:H] / o[..., H : H + 1]


def _run(x, Wq, Wk, Wv, trace=False):
    nc = _get_nc()
    in_maps = _prep_inputs(x, Wq, Wk, Wv)
    res = run_bass_kernel_spmd(nc, in_maps, list(range(NCORES)), trace=trace)
    out = np.concatenate(
        [_unstage(res.results[i]["out"]) for i in range(NCORES)], axis=0
    )
    return np.ascontiguousarray(out.astype(np.float32)), res


def kernel(x, Wq, Wk, Wv):
    out, _ = _run(x, Wq, Wk, Wv, trace=False)
    return out
